# revision 17
# baseline (speedup 1.0000x reference)
"""Trainium2 Bass kernel for gated sparse attention (B=4,S=2048,E=256,H=8,C=32).

Sharding: 8 cores <- (batch, row-half) pairs. Each core computes all heads for
its 1024 query rows; keys/values span the full 2048 rows of its batch.
"""

import functools
import math
import sys

sys.path.insert(0, "/opt/trn_rl_repo")

import numpy as np

B, S, E = 4, 2048, 256
H, C = 8, 32
SL = S // 2  # local query rows per core
EPS = 1e-5
NCORES = 8
SCALE = 1.0 / math.sqrt(C)


def _build_kernel():
    import concourse.bass as bass  # noqa: F401
    import concourse.mybir as mybir
    import concourse.tile as tile
    from concourse import bacc
    from concourse.masks import make_identity
    from contextlib import ExitStack

    f32 = mybir.dt.float32
    f32r = mybir.dt.float32r
    bf16 = mybir.dt.bfloat16
    i32 = mybir.dt.int32
    import os
    ET_DT = bf16 if os.environ.get("ET_BF16") else f32r

    def R(ap):
        return ap.bitcast(f32r) if ap.dtype == f32 else ap
    AF = mybir.ActivationFunctionType
    ALU = mybir.AluOpType

    nc = bacc.Bacc("TRN2", target_bir_lowering=False, debug=False, num_devices=NCORES)

    xf_d = nc.declare_dram_parameter("xf", [S, E], f32, isOutput=False)
    xl_d = nc.declare_dram_parameter("xl", [SL, E], f32, isOutput=False)
    # bias is passed pre-transposed by the host: [h, j, i]
    bias_d = nc.declare_dram_parameter("bias", [H, S, SL], f32r, isOutput=False)
    mask_d = nc.declare_dram_parameter("mask", [S], i32, isOutput=False)
    wq_d = nc.declare_dram_parameter("wq", [E, E], f32r, isOutput=False)
    wk_d = nc.declare_dram_parameter("wk", [E, E], f32r, isOutput=False)
    wv_d = nc.declare_dram_parameter("wv", [E, E], f32r, isOutput=False)
    wg_d = nc.declare_dram_parameter("wg", [E, E], f32r, isOutput=False)
    bg_d = nc.declare_dram_parameter("bg", [E], f32, isOutput=False)
    wo_d = nc.declare_dram_parameter("wo", [E, E], f32r, isOutput=False)
    bo_d = nc.declare_dram_parameter("bo", [E], f32r, isOutput=False)
    out_d = nc.declare_dram_parameter("out", [SL, E], f32, isOutput=True)

    NB_F = S // 128   # 16 row blocks (full)
    NB_L = SL // 128  # 8 row blocks (local)
    NJB = S // 128    # 16 key blocks
    NIC = 2           # i-chunks of 512 local rows

    with tile.TileContext(nc, pool_alloc_mode="queue") as tc, ExitStack() as ctx:
        const = ctx.enter_context(tc.tile_pool(name="const", bufs=1))
        persist = ctx.enter_context(tc.tile_pool(name="persist", bufs=1))

        ident = const.tile([128, 128], f32)
        make_identity(nc, ident)
        identr = const.tile([128, 128], f32r)
        nc.vector.tensor_copy(identr[:], ident[:])
        ones_row = const.tile([1, 128], f32)
        nc.vector.memset(ones_row[:], 1.0)
        ones_row_r = const.tile([1, 128], f32r)
        nc.vector.tensor_copy(ones_row_r[:], ones_row[:])
        ones8 = const.tile([128, 8], f32)
        nc.vector.memset(ones8[:], 1.0)

        # weights: [e, hc] -> [p, ec, hc] (lhsT chunks along contraction dim e)
        wq_sb = const.tile([128, 2, E], f32r)
        nc.sync.dma_start(out=wq_sb[:], in_=wq_d.rearrange("(ec p) m -> p ec m", p=128))
        wk_sb = const.tile([128, 2, E], f32r)
        nc.sync.dma_start(out=wk_sb[:], in_=wk_d.rearrange("(ec p) m -> p ec m", p=128))
        wv_sb = const.tile([128, 2, E], f32r)
        nc.sync.dma_start(out=wv_sb[:], in_=wv_d.rearrange("(ec p) m -> p ec m", p=128))
        wg_sb = const.tile([128, 2, E], f32r)
        nc.sync.dma_start(out=wg_sb[:], in_=wg_d.rearrange("(ec p) m -> p ec m", p=128))
        # wo: [hc, e] -> [c, h, e] so per-head rhs slices sit at partition 0
        wo_sb = const.tile([32, 8, E], f32r)
        nc.sync.dma_start(out=wo_sb[:], in_=wo_d.rearrange("(h c) e -> c h e", c=32))
        bg_sb = const.tile([32, 8], f32)
        nc.sync.dma_start(out=bg_sb[:], in_=bg_d.rearrange("(h c) -> c h", c=32))
        bo_sb = const.tile([1, E], f32r)
        nc.sync.dma_start(out=bo_sb[:], in_=bo_d.rearrange("(o e) -> o e", o=1))
        mask_sb = const.tile([128, 16], i32)
        nc.sync.dma_start(out=mask_sb[:], in_=mask_d.rearrange("(jb p) -> p jb", p=128))
        maskf = const.tile([128, 16], f32)
        nc.vector.tensor_copy(maskf[:], mask_sb[:])

        # xn (layernormed x) in natural layout, then transposed
        xnTf = persist.tile([128, 2, S], f32r)   # [e%128, ec, j]
        xnTl = persist.tile([128, 2, SL], f32r)  # [e%128, ec, i]
        vp = persist.tile([128, NJB, 8 * 33], ET_DT)  # [j%128, jb, (h, c+mask)]
        gav = persist.tile([32, 8, SL], f32r)    # gated+normalized avT [c, h, i]

        with tc.tile_pool(name="xraw", bufs=1) as xraw, \
             tc.tile_pool(name="lnscr", bufs=3) as lnscr, \
             tc.tile_pool(name="lnst", bufs=4) as lnst, \
             tc.tile_pool(name="xn", bufs=1) as xnpool, \
             tc.tile_pool(name="tpsum", bufs=2, space="PSUM") as tpsum:
            xf_sb = xraw.tile([128, NB_F, E], f32)
            nc.sync.dma_start(out=xf_sb[:], in_=xf_d.rearrange("(b p) e -> p b e", p=128))
            xl_sb = xraw.tile([128, NB_L, E], f32)
            nc.sync.dma_start(out=xl_sb[:], in_=xl_d.rearrange("(b p) e -> p b e", p=128))

            xnf = xnpool.tile([128, NB_F, E], f32)
            xnl = xnpool.tile([128, NB_L, E], f32)

            def ln_block(src, dst):
                rs = lnst.tile([128, 1], f32, tag="rs")
                nc.vector.reduce_sum(rs[:], src, axis=mybir.AxisListType.X)
                negmean = lnst.tile([128, 1], f32, tag="nm")
                nc.vector.tensor_scalar_mul(negmean[:], rs[:], -1.0 / E)
                xc = lnscr.tile([128, E], f32, tag="xc")
                nc.vector.tensor_scalar_add(xc[:], src, negmean[:])
                sq = lnscr.tile([128, E], f32, tag="sq")
                ssq = lnst.tile([128, 1], f32, tag="ssq")
                nc.scalar.activation(sq[:], xc[:], AF.Square, accum_out=ssq[:])
                var = lnst.tile([128, 1], f32, tag="var")
                nc.vector.tensor_scalar(
                    var[:], ssq[:], 1.0 / E, EPS, op0=ALU.mult, op1=ALU.add
                )
                sd = lnst.tile([128, 1], f32, tag="sd")
                nc.scalar.sqrt(sd[:], var[:])
                rstd = lnst.tile([128, 1], f32, tag="rstd")
                nc.vector.reciprocal(rstd[:], sd[:])
                nc.vector.tensor_scalar_mul(dst, xc[:], rstd[:])

            for b in range(NB_F):
                ln_block(xf_sb[:, b, :], xnf[:, b, :])
            for b in range(NB_L):
                ln_block(xl_sb[:, b, :], xnl[:, b, :])

            # transpose xn -> xnT via PE (128x128 blocks)
            for ec in range(2):
                for nb in range(4):  # full: 16 blocks -> 4 psum tiles of [128, 512]
                    tp = tpsum.tile([128, 512], f32, tag="tp")
                    for tb in range(4):
                        nc.tensor.matmul(
                            out=tp[:, tb * 128:(tb + 1) * 128],
                            lhsT=xnf[:, nb * 4 + tb, ec * 128:(ec + 1) * 128],
                            rhs=ident[:],
                            is_transpose=True,
                            start=(tb == 0),
                            stop=(tb == 3),
                            skip_group_check=True,
                        )
                    nc.vector.tensor_copy(xnTf[:, ec, nb * 512:(nb + 1) * 512], tp[:])
                for nb in range(2):  # local: 8 blocks -> 2 psum tiles
                    tp = tpsum.tile([128, 512], f32, tag="tp")
                    for tb in range(4):
                        nc.tensor.matmul(
                            out=tp[:, tb * 128:(tb + 1) * 128],
                            lhsT=xnl[:, nb * 4 + tb, ec * 128:(ec + 1) * 128],
                            rhs=ident[:],
                            is_transpose=True,
                            start=(tb == 0),
                            stop=(tb == 3),
                            skip_group_check=True,
                        )
                    nc.vector.tensor_copy(xnTl[:, ec, nb * 512:(nb + 1) * 512], tp[:])

            # v projection (natural layout) + mask fold + mask column
            with tc.tile_pool(name="vpsum", bufs=2, space="PSUM") as vpsum:
                for jb in range(NJB):
                    pv = vpsum.tile([128, E], f32)
                    for ec in range(2):
                        nc.tensor.matmul(
                            out=pv[:],
                            lhsT=R(xnTf[:, ec, jb * 128:(jb + 1) * 128]),
                            rhs=R(wv_sb[:, ec, :]),
                            start=(ec == 0),
                            stop=(ec == 1),
                        )
                    vslab = vp[:, jb, :].rearrange("p (h cc) -> p h cc", cc=33)
                    nc.vector.tensor_scalar_mul(
                        vslab[:, :, 0:32],
                        pv[:].rearrange("p (h c) -> p h c", c=32),
                        maskf[:, jb:jb + 1],
                    )
                    nc.vector.tensor_scalar_mul(
                        vslab[:, :, 32:33],
                        ones8[:].rearrange("p (h o) -> p h o", o=1),
                        maskf[:, jb:jb + 1],
                    )

        # main attention loop
        with tc.tile_pool(name="biasp", bufs=2) as biasp, \
             tc.tile_pool(name="qkg", bufs=2) as qkg, \
             tc.tile_pool(name="epool", bufs=3) as epool, \
             tc.tile_pool(name="rpool", bufs=2) as rpool, \
             tc.tile_pool(name="gtmp", bufs=2) as gtmp, \
             tc.tile_pool(name="pjp", bufs=2, space="PSUM") as pjp, \
             tc.tile_pool(name="stp", bufs=3, space="PSUM") as stp, \
             tc.tile_pool(name="avp", bufs=2, space="PSUM") as avp, \
             tc.tile_pool(name="rp", bufs=1, space="PSUM") as rp:

            def emit_projections(h):
                # per-head projections: qT/gateT [32, SL], kT [32, S]
                # psum chunks are [32, 512] (one bank) so pjp stays at 2 banks
                qT = qkg.tile([32, SL], f32r, tag="qT")
                for nb in range(2):
                    pq = pjp.tile([32, 512], f32, tag="pj")
                    for ec in range(2):
                        nc.tensor.matmul(
                            out=pq[:],
                            lhsT=R(wq_sb[:, ec, h * 32:(h + 1) * 32]),
                            rhs=R(xnTl[:, ec, nb * 512:(nb + 1) * 512]),
                            start=(ec == 0),
                            stop=(ec == 1),
                        )
                    nc.vector.tensor_scalar_mul(
                        qT[:, nb * 512:(nb + 1) * 512], pq[:], SCALE
                    )
                gT = qkg.tile([32, SL], f32r, tag="gT")
                for nb in range(2):
                    pg = pjp.tile([32, 512], f32, tag="pj")
                    for ec in range(2):
                        nc.tensor.matmul(
                            out=pg[:],
                            lhsT=R(wg_sb[:, ec, h * 32:(h + 1) * 32]),
                            rhs=R(xnTl[:, ec, nb * 512:(nb + 1) * 512]),
                            start=(ec == 0),
                            stop=(ec == 1),
                        )
                    nc.scalar.activation(
                        gT[:, nb * 512:(nb + 1) * 512], pg[:], AF.Sigmoid,
                        bias=bg_sb[:, h:h + 1],
                    )
                kT = qkg.tile([32, S], f32r, tag="kT")
                for nb in range(4):
                    pk = pjp.tile([32, 512], f32, tag="pj")
                    for ec in range(2):
                        nc.tensor.matmul(
                            out=pk[:],
                            lhsT=R(wk_sb[:, ec, h * 32:(h + 1) * 32]),
                            rhs=R(xnTf[:, ec, nb * 512:(nb + 1) * 512]),
                            start=(ec == 0),
                            stop=(ec == 1),
                        )
                    nc.vector.tensor_copy(kT[:, nb * 512:(nb + 1) * 512], pk[:])
                return qT, gT, kT

            proj = emit_projections(0)
            pending_tail = None
            for h in range(H):
                qT, gT, kT = proj
                for ic in range(NIC):
                    bias_t = biasp.tile([128, NJB, 512], f32r, tag="bias")
                    nc.sync.dma_start(
                        out=bias_t[:],
                        in_=bias_d[h, :, ic * 512:(ic + 1) * 512].rearrange(
                            "(jb p) i -> p jb i", p=128
                        ),
                    )
                    av = avp.tile([33, 512], f32, tag="av")
                    # software pipeline: PV for jb runs one iteration late so
                    # the in-order PE never stalls waiting on ACT's exp.
                    pending = None
                    for jb in range(NJB):
                        st = stp.tile([128, 512], f32, tag="st")
                        nc.tensor.matmul(
                            out=st[:],
                            lhsT=identr[:],
                            rhs=R(bias_t[:, jb, :]),
                            start=True,
                            stop=False,
                        )
                        nc.tensor.matmul(
                            out=st[:],
                            lhsT=R(kT[:, jb * 128:(jb + 1) * 128]),
                            rhs=R(qT[:, ic * 512:(ic + 1) * 512]),
                            start=False,
                            stop=True,
                        )
                        et = epool.tile([128, 512], ET_DT, tag="et")
                        nc.scalar.activation(et[:], st[:], AF.Exp)
                        if pending is not None:
                            pjb, pet = pending
                            nc.tensor.matmul(
                                out=av[:],
                                lhsT=R(vp[:, pjb, h * 33:(h + 1) * 33]),
                                rhs=R(pet[:]),
                                start=(pjb == 0),
                                stop=False,
                            )
                        pending = (jb, et)
                        if jb == 3 and pending_tail is not None:
                            pending_tail()
                            pending_tail = None
                    pjb, pet = pending
                    nc.tensor.matmul(
                        out=av[:],
                        lhsT=R(vp[:, pjb, h * 33:(h + 1) * 33]),
                        rhs=R(pet[:]),
                        start=False,
                        stop=True,
                    )
                    if h + 1 < H and ic == 0:
                        proj = emit_projections(h + 1)

                    def make_tail(av=av, gT=gT, h=h, ic=ic):
                        def tail():
                            # av rows 0-31 = unnorm attn@v ; row 32 = sum(exp)
                            rins = rpool.tile([33, 544], f32, tag="rins")
                            nc.vector.reciprocal(rins[32:33, 0:512], av[32:33, :])
                            nc.vector.memset(rins[32:33, 512:544], 1.0)
                            rpt = rp.tile([32, 512], f32, tag="rpt")
                            nc.tensor.matmul(
                                out=rpt[:],
                                lhsT=rins[32:33, 512:544],
                                rhs=rins[32:33, 0:512],
                                tile_position=(32, 0),
                            )
                            t1 = gtmp.tile([32, 512], f32, tag="t1")
                            nc.vector.tensor_mul(
                                t1[:], av[0:32, :], gT[:, ic * 512:(ic + 1) * 512]
                            )
                            nc.vector.tensor_mul(
                                gav[:, h, ic * 512:(ic + 1) * 512], t1[:], rpt[:]
                            )
                        return tail

                    pending_tail = make_tail()
            pending_tail()
            pending_tail = None

        # final projection: out[i, e] = sum_h gav_h^T @ wo_h + bo
        with tc.tile_pool(name="ops", bufs=2, space="PSUM") as ops, \
             tc.tile_pool(name="osb", bufs=2) as osb:
            for ib in range(NB_L):
                po = ops.tile([128, E], f32)
                for h in range(H):
                    nc.tensor.matmul(
                        out=po[:],
                        lhsT=R(gav[:, h, ib * 128:(ib + 1) * 128]),
                        rhs=R(wo_sb[:, h, :]),
                        start=(h == 0),
                        stop=False,
                    )
                nc.tensor.matmul(
                    out=po[:], lhsT=ones_row_r[:], rhs=bo_sb[:], start=False, stop=True
                )
                ot = osb.tile([128, E], f32)
                nc.vector.tensor_copy(ot[:], po[:])
                nc.sync.dma_start(out=out_d[ib * 128:(ib + 1) * 128, :], in_=ot[:])

    nc.compile()
    return nc


@functools.lru_cache(maxsize=1)
def _get_nc():
    return _build_kernel()


def _make_in_maps(x, mask, attn_bias, wq, wk, wv, wg, bg, wo, bo):
    in_maps = []
    for core in range(NCORES):
        b, half = core // 2, core % 2
        lo = half * SL
        in_maps.append({
            "xf": np.ascontiguousarray(x[b]),
            "xl": np.ascontiguousarray(x[b, lo:lo + SL]),
            "bias": np.ascontiguousarray(
                attn_bias[b, :, lo:lo + SL, :].transpose(0, 2, 1)
            ),
            "mask": np.ascontiguousarray(mask[b]),
            "wq": np.ascontiguousarray(wq),
            "wk": np.ascontiguousarray(wk),
            "wv": np.ascontiguousarray(wv),
            "wg": np.ascontiguousarray(wg),
            "bg": np.ascontiguousarray(bg),
            "wo": np.ascontiguousarray(wo),
            "bo": np.ascontiguousarray(bo),
        })
    return in_maps


def kernel(x, mask, attn_bias, wq, wk, wv, wg, bg, wo, bo, _trace=False):
    x = np.asarray(x, dtype=np.float32)
    mask = np.asarray(mask, dtype=np.int32)
    attn_bias = np.asarray(attn_bias, dtype=np.float32)
    wq, wk, wv, wg = (np.asarray(a, dtype=np.float32) for a in (wq, wk, wv, wg))
    bg = np.asarray(bg, dtype=np.float32)
    wo = np.asarray(wo, dtype=np.float32)
    bo = np.asarray(bo, dtype=np.float32)

    from concourse.bass_utils import run_bass_kernel_spmd

    nc = _get_nc()
    in_maps = _make_in_maps(x, mask, attn_bias, wq, wk, wv, wg, bg, wo, bo)
    res = run_bass_kernel_spmd(nc, in_maps, list(range(NCORES)), trace=_trace)

    out = np.empty((B, S, E), dtype=np.float32)
    for core in range(NCORES):
        b, half = core // 2, core % 2
        out[b, half * SL:(half + 1) * SL] = res.results[core]["out"]
    if _trace:
        return out, res
    return out
